# revision 25
# baseline (speedup 1.0000x reference)
"""Trainium2 Bass kernel for an AttentionBlock (1x1-conv QKV + softmax attention + residual).

Reference computation (per batch b):
    q = Wq@x + bq  [32, N];  k = Wk@x + bk  [32, N];  v = Wv@x + bv  [256, N]
    attn = softmax_j(q_i . k_j);  out[c, i] = sum_j v[c, j] attn[i, j]
    final = gamma * out + x            (N = 64*64 = 4096)

Sharding: 8 cores = 4 batches x 2 query-halves (2048 queries per core).
Each core receives x[b] with its columns rolled so its own query half sits at
columns 0:2048 (softmax is invariant to a permutation of the key/value axis).

Per-core device program (all matmul operands bf16 so the PE streams one
column per cycle; PSUM accumulation stays fp32):
    dummy matmuls on a memset tile warm the HAM clock gate while x loads
    x arrives bf16 in 8 column chunks over the three DMA rings
    per chunk the PE projects (psum->sbuf copies balanced vector/scalar):
      k_sb [128, 4096]  = Wk4.T @ x (+bk4)  - Wk tiled 4x on the host so one
                          matmul writes all four 32-row replicas the packed
                          score matmuls need
      q_sb [128, 2048]  = Wq4.T @ x[:, 0:2048] (+bq4)
      vT_sb [4096, 257] = x.T @ (gamma WvT); col 256 = 1.0 (memset) for the
                          softmax denominator
    scoresT[j, i]       = k-tile.T @ q           (PE, bf16, quadrant-packed)
    e = exp(scoresT-40) PSUM -> SBUF bf16        (ACT; shift avoids overflow)
    out[i, 0:257]      += e-chunk.T @ vT-tile    (PE, bf16)
    final[i, c] = out[i, c]/denom + xr[i, c]     (fused DVE op; xr = x.T +
                  gamma*bv precomputed on the host absorbs the V bias, and
                  gamma rides in WvT, so no extra scale is needed)
Output is stored [n, c]; the host transposes back to [c, n].
"""

import sys

if "/opt/trn_rl_repo" not in sys.path:
    sys.path.insert(0, "/opt/trn_rl_repo")

import numpy as np

import concourse.bass as bass
import concourse.tile as tile
from concourse import bacc
from concourse import mybir

F32 = mybir.dt.float32
BF16 = mybir.dt.bfloat16

C = 256          # channels
D = 32           # q/k channels
NK = 4096        # keys per core (full sequence)
NQ = 2048        # queries per core (half sequence)
NJ = NK // 128   # 32 key tiles
NG = 4           # query groups
GI = 4           # i-tiles (128 queries) per group
ISPAN = NQ // NG  # 512 query columns per group
NCH = 8          # x arrives in 8 chunks of 512 columns
CHW = NK // NCH
EXP_SHIFT = -40.0

Exp = mybir.ActivationFunctionType.Exp
Copy = mybir.ActivationFunctionType.Copy
MULT = mybir.AluOpType.mult
ADD = mybir.AluOpType.add


def build(nc):
    x_bf = nc.declare_dram_parameter("x_bf", [C, NK], BF16, isOutput=False)
    xqT = nc.declare_dram_parameter("xqT", [NQ, C], F32, isOutput=False)
    # projection weights pre-arranged in SBUF layout [p, h, :] so each load
    # is one DMA with contiguous lines; wk separate (it gates the first
    # projection and rides the fast sync ring), wq|gamma*WvT packed on gpsimd
    wkp = nc.declare_dram_parameter("wkp", [128, 2, 128], BF16, isOutput=False)
    wqv = nc.declare_dram_parameter("wqv", [128, 2, 384], BF16, isOutput=False)
    bpack = nc.declare_dram_parameter("bpack", [128, 2], F32, isOutput=False)
    out_nc = nc.declare_dram_parameter("out_nc", [NQ, C], F32, isOutput=True)

    with tile.TileContext(nc) as tc:
        with (
            tc.tile_pool(name="singles", bufs=1) as singles,
            tc.tile_pool(name="epool", bufs=3) as e_pool,
            tc.tile_pool(name="osb", bufs=4) as osb_pool,
            tc.tile_pool(name="small", bufs=8) as small_pool,
            tc.tile_pool(name="s_ps", bufs=2, space="PSUM") as s_pool,
            tc.tile_pool(name="o_ps", bufs=4, space="PSUM") as o_pool,
        ):
            # PE warmup: the HAM clock gate keeps the tensor engine at half
            # clock until ~3.4us of sustained activity. Burn that window on
            # dummy matmuls over a memset tile while the x DMA is in flight
            # so the real projections run at full clock.
            wu_sb = singles.tile([128, 256], BF16)
            nc.gpsimd.memset(wu_sb, 0.0)
            for _ in range(16):
                ps_w = s_pool.tile([128, 128], F32, tag="ps_s", name="ps_wu")
                nc.tensor.matmul(
                    ps_w, wu_sb[:, 0:128], wu_sb[:, 0:128], start=True, stop=True
                )

            # ---------------- persistent SBUF inputs ----------------
            # wk leads the sync ring, then x chunks rotate over the
            # sync/scalar/gpsimd rings so all three drain in parallel. xqT
            # (residual, fp32) is only needed at the epilogues and queues
            # last on gpsimd.
            xf_sb = singles.tile([128, 2, NK], BF16)
            x_r = x_bf.rearrange("(h p) n -> p h n", p=128)

            def xsl(nch):
                return slice(nch * CHW, (nch + 1) * CHW)

            wk_sb = singles.tile([128, 2, 128], BF16)
            wqv_sb = singles.tile([128, 2, 384], BF16)
            bp_sb = singles.tile([128, 2], F32)
            nc.scalar.dma_start(out=wk_sb, in_=wkp[:, :, :])
            nc.gpsimd.dma_start(out=wqv_sb, in_=wqv[:, :, :])
            nc.gpsimd.dma_start(out=bp_sb, in_=bpack[:, :])
            for nch, eng in [
                (0, nc.sync), (1, nc.scalar), (6, nc.gpsimd), (2, nc.sync),
                (3, nc.scalar), (7, nc.gpsimd), (4, nc.sync), (5, nc.scalar),
            ]:
                eng.dma_start(out=xf_sb[:, :, xsl(nch)], in_=x_r[:, :, xsl(nch)])
            wq_sb = wqv_sb[:, :, 0:128]
            wv_sb = wqv_sb[:, :, 128:384]
            bk_sb = bp_sb[:, 0:1]
            bq_sb = bp_sb[:, 1:2]

            xqT_sb = singles.tile([128, NQ // 128, C], F32)
            nc.gpsimd.dma_start(
                out=xqT_sb, in_=xqT.rearrange("(t p) c -> p t c", p=128)
            )

            shift_sb = singles.tile([128, 1], F32)
            nc.vector.memset(shift_sb, EXP_SHIFT)

            # ---------------- projections (chunk-pipelined) ----------------
            k_sb = singles.tile([128, NK], BF16)
            q_sb = singles.tile([128, NQ], BF16)
            vT_sb = singles.tile([128, NJ, C + 1], BF16)
            nc.vector.memset(vT_sb[:, :, C : C + 1], 1.0)

            for ch in range(NCH):
                ps = s_pool.tile([128, CHW], F32, tag="ps_s", name="ps_kq")
                for h in range(2):
                    nc.tensor.matmul(
                        ps,
                        wk_sb[:, h, :],
                        xf_sb[:, h, xsl(ch)],
                        start=(h == 0),
                        stop=(h == 1),
                    )
                if ch % 2 == 0:
                    nc.vector.tensor_scalar_add(k_sb[:, xsl(ch)], ps, bk_sb)
                else:
                    nc.scalar.add(k_sb[:, xsl(ch)], ps, bk_sb)

                if ch < NQ // CHW:
                    ps = s_pool.tile([128, CHW], F32, tag="ps_s", name="ps_kq")
                    for h in range(2):
                        nc.tensor.matmul(
                            ps,
                            wq_sb[:, h, :],
                            xf_sb[:, h, xsl(ch)],
                            start=(h == 0),
                            stop=(h == 1),
                        )
                    if ch % 2 == 0:
                        nc.scalar.add(q_sb[:, xsl(ch)], ps, bq_sb)
                    else:
                        nc.vector.tensor_scalar_add(q_sb[:, xsl(ch)], ps, bq_sb)

                for j in range(ch * 4, ch * 4 + 4):
                    psv = o_pool.tile([128, C], F32, tag="ps_o", name="ps_v")
                    for h in range(2):
                        nc.tensor.matmul(
                            psv,
                            xf_sb[:, h, j * 128 : (j + 1) * 128],
                            wv_sb[:, h, :],
                            start=(h == 0),
                            stop=(h == 1),
                        )
                    # psum->sbuf copies split between vector and scalar so
                    # neither trails the PE during the projection phase
                    if j % 2 == 0:
                        nc.vector.tensor_copy(vT_sb[:, j, 0:C], psv)
                    else:
                        nc.scalar.mul(vT_sb[:, j, 0:C], psv, 1.0)

            # ---------------- attention ----------------
            # Flat software pipeline over (group, quad) steps at half-quad
            # granularity: while ACT runs exp on one half, the PE issues the
            # next step's packed score matmuls, then consumes the current
            # half with 8 attn matmuls. Scores prefetch crosses group
            # boundaries so the PE never drains at an epilogue.
            steps = [(g, q4) for g in range(NG) for q4 in range(NJ // 4)]
            score_tiles = {}
            ps_o_groups = {}

            def emit_scores_half(step, half):
                g, q4 = step
                isl = slice(g * ISPAN, (g + 1) * ISPAN)
                ps_s = s_pool.tile([128, 2, ISPAN], F32, tag="ps_s", name="ps_s")
                for rr in range(2):
                    r = half * 2 + rr
                    j = q4 * 4 + r
                    nc.tensor.matmul(
                        ps_s[:, rr, :],
                        k_sb[32 * r : 32 * (r + 1), j * 128 : (j + 1) * 128],
                        q_sb[32 * r : 32 * (r + 1), isl],
                        start=True,
                        stop=True,
                        tile_position=(32 * r, 0),
                    )
                score_tiles[(g, q4, half)] = ps_s

            emit_scores_half(steps[0], 0)
            emit_scores_half(steps[0], 1)
            for idx, (g, q4) in enumerate(steps):
                if q4 == 0:
                    ps_o_groups[g] = [
                        o_pool.tile([128, C + 1], F32, tag="ps_o", name="ps_o")
                        for _ in range(GI)
                    ]
                ps_o = ps_o_groups[g]
                nxt = steps[idx + 1] if idx + 1 < len(steps) else None
                for half in range(2):
                    ps_s = score_tiles.pop((g, q4, half))
                    e_sb = e_pool.tile(
                        [128, 2, ISPAN], BF16, tag="e_sb", name="e_sb"
                    )
                    nc.scalar.activation(
                        e_sb, ps_s, Exp, bias=shift_sb, scale=1.0
                    )
                    if nxt is not None:
                        emit_scores_half(nxt, half)
                    for rr in range(2):
                        r = half * 2 + rr
                        j = q4 * 4 + r
                        for t in range(GI):
                            nc.tensor.matmul(
                                ps_o[t],
                                e_sb[:, rr, t * 128 : (t + 1) * 128],
                                vT_sb[:, j, :],
                                start=(j == 0),
                                stop=(j == NJ - 1),
                            )
                if q4 == NJ // 4 - 1:
                    # Epilogue: f = out/denom + xr (gamma is folded into the
                    # host-side WvT and xr, so out already carries it).
                    # Reciprocals first on vector, then the wide mul+add work
                    # fans across vector (fused) and scalar+gpsimd so the
                    # psum buffers free fast and the last group drains fast.
                    # Everything rides vector here: an epilogue op on scalar
                    # would delay the next group's exp behind it in the FIFO.
                    for t in range(GI):
                        it = g * GI + t
                        r = small_pool.tile([128, 1], F32, tag="r", name="r")
                        nc.vector.reciprocal(r, ps_o[t][:, C : C + 1])
                        f_sb = osb_pool.tile([128, C], F32, tag="f_sb", name="f_sb")
                        nc.vector.scalar_tensor_tensor(
                            f_sb, ps_o[t][:, 0:C], r, xqT_sb[:, it, :],
                            op0=MULT, op1=ADD,
                        )
                        orows = out_nc[it * 128 : (it + 1) * 128, :]
                        if t < 3:
                            eng = (nc.sync, nc.scalar, nc.gpsimd)[t]
                            eng.dma_start(out=orows, in_=f_sb)
                        else:
                            nc.sync.dma_start(
                                out=orows[0:64, :], in_=f_sb[0:64, :]
                            )
                            nc.scalar.dma_start(
                                out=orows[64:128, :], in_=f_sb[64:128, :]
                            )
                    del ps_o_groups[g]
    return nc


def _install_trace_support():
    """Profiling-only plumbing for KERNEL_TRACE=1 runs: register the NTFF
    profile hook (this image's antenv lacks the axon_hooks shim) and keep
    trace artifacts local instead of uploading. Never used in plain runs."""
    import importlib.util
    import types

    import concourse.bass_utils as bu

    bu.upload_artifacts = lambda tmpdir: tmpdir
    if "antenv.axon_hooks" in sys.modules:
        return
    try:
        if importlib.util.find_spec("antenv.axon_hooks") is not None:
            return
    except (ValueError, ModuleNotFoundError):
        return
    import antenv
    from trn_agent_boot.trn_boot import _ntff_profile_via_ctypes

    mod = types.ModuleType("antenv.axon_hooks")
    mod._hook = _ntff_profile_via_ctypes("/opt/axon/libaxon_pjrt.so")
    mod.set_axon_ntff_profile_hook = lambda h: setattr(mod, "_hook", h)
    mod.get_axon_ntff_profile_hook = lambda: mod._hook
    sys.modules["antenv.axon_hooks"] = mod
    antenv.axon_hooks = mod


_cached = None


def _get_module():
    global _cached
    if _cached is None:
        nc = bacc.Bacc()
        build(nc)
        if not nc.is_finalized():
            nc.finalize()
        _cached = nc
    return _cached


def kernel(x, Wq, bq, Wk, bk, Wv, bv, gamma, **_unused):
    from concourse.bass_utils import run_bass_kernel_spmd
    import os

    B, Cx, H, W = x.shape
    N = H * W
    xf = np.ascontiguousarray(np.asarray(x, dtype=np.float32).reshape(B, Cx, N))
    Wq = np.asarray(Wq, np.float32)
    Wk = np.asarray(Wk, np.float32)
    Wv = np.asarray(Wv, np.float32)
    bq = np.asarray(bq, np.float32)
    bk = np.asarray(bk, np.float32)
    bv = np.asarray(bv, np.float32)
    gamma = np.asarray(gamma, np.float32)

    import ml_dtypes

    bf16 = ml_dtypes.bfloat16
    # weight pack in device SBUF layout [p, h, wk4 | wq4 | gamma*WvT]: Wk/Wq
    # tiled 4x along the output dim so one matmul writes all four 32-row
    # replicas; gamma folded into Wv so the epilogue needs no extra scale
    wkp = np.ascontiguousarray(
        np.tile(Wk.T, (1, 4)).reshape(2, 128, 128).transpose(1, 0, 2).astype(bf16)
    )
    wqv_cat = np.concatenate([np.tile(Wq.T, (1, 4)), gamma[0] * Wv.T], axis=1)
    wqv = np.ascontiguousarray(
        wqv_cat.reshape(2, 128, 384).transpose(1, 0, 2).astype(bf16)
    )
    bpack = np.ascontiguousarray(
        np.stack([np.tile(bk, 4), np.tile(bq, 4)], axis=1)
    )

    in_maps = []
    for core in range(8):
        b, half = core // 2, core % 2
        ioff = half * NQ
        xb = xf[b]
        x_bf = np.ascontiguousarray(np.roll(xb, -ioff, axis=1).astype(bf16))
        # residual with the V-projection bias folded in: the device computes
        # (e @ (gamma*v0))/denom + xqT, and gamma * e @ (v0 + bv) / denom
        # collapses to an extra gamma*bv[c] term
        xqT_np = np.ascontiguousarray(
            xb[:, ioff : ioff + NQ].T + gamma[0] * bv[None, :]
        )
        in_maps.append(
            {
                "x_bf": x_bf,
                "xqT": xqT_np,
                "wkp": wkp,
                "wqv": wqv,
                "bpack": bpack,
            }
        )

    nc = _get_module()
    trace = bool(int(os.environ.get("KERNEL_TRACE", "0")))
    if trace:
        _install_trace_support()
        tmpdir = os.environ.get("KERNEL_TRACE_DIR") or None
        res = run_bass_kernel_spmd(
            nc, in_maps, core_ids=list(range(8)), trace=True, tmpdir=tmpdir
        )
    else:
        res = run_bass_kernel_spmd(nc, in_maps, core_ids=list(range(8)))
    if trace and res.exec_time_ns is not None:
        print(f"HW exec time: {res.exec_time_ns} ns")
        print(f"HW exec time mean: {res.mean_exec_time_ns} ns")
        if res.instructions_and_trace is not None:
            print(f"trace: {res.instructions_and_trace[1]}")

    out = np.empty((B, Cx, N), np.float32)
    for core in range(8):
        b, half = core // 2, core % 2
        out[b][:, half * NQ : (half + 1) * NQ] = res.results[core]["out_nc"].T
    return out.reshape(B, Cx, H, W)


# revision 28
# speedup vs baseline: 1.1658x; 1.1658x over previous
"""Trainium2 Bass kernel for an AttentionBlock (1x1-conv QKV + softmax attention + residual).

Reference computation (per batch b):
    q = Wq@x + bq  [32, N];  k = Wk@x + bk  [32, N];  v = Wv@x + bv  [256, N]
    attn = softmax_j(q_i . k_j);  out[c, i] = sum_j v[c, j] attn[i, j]
    final = gamma * out + x            (N = 64*64 = 4096)

Sharding: 8 cores = 4 batches x 2 query-halves (2048 queries per core).
Each core receives x[b] with its columns rolled so its own query half sits at
columns 0:2048 (softmax is invariant to a permutation of the key/value axis).

Per-core device program (all matmul operands bf16 so the PE streams one
column per cycle; PSUM accumulation stays fp32):
    dummy matmuls on a memset tile warm the HAM clock gate while x loads
    x arrives bf16 in 8 column chunks over the three DMA rings
    per chunk the PE projects (psum->sbuf copies balanced vector/scalar):
      k_sb [128, 4096]  = Wk4.T @ x (+bk4)  - Wk tiled 4x on the host so one
                          matmul writes all four 32-row replicas the packed
                          score matmuls need
      q_sb [128, 2048]  = Wq4.T @ x[:, 0:2048] (+bq4)
      vT_sb [4096, 257] = x.T @ (gamma WvT); col 256 = 1.0 (memset) for the
                          softmax denominator
    scoresT[j, i]       = k-tile.T @ q           (PE, bf16, quadrant-packed)
    e = exp(scoresT-40) PSUM -> SBUF bf16        (ACT; shift avoids overflow)
    out[i, 0:257]      += e-chunk.T @ vT-tile    (PE, bf16)
    final[i, c] = out[i, c]/denom + xr[i, c]     (fused DVE op; xr = x.T +
                  gamma*bv precomputed on the host absorbs the V bias, and
                  gamma rides in WvT, so no extra scale is needed)
Output is stored [n, c]; the host transposes back to [c, n].
"""

import sys

if "/opt/trn_rl_repo" not in sys.path:
    sys.path.insert(0, "/opt/trn_rl_repo")

import numpy as np

import concourse.bass as bass
import concourse.tile as tile
from concourse import bacc
from concourse import mybir

F32 = mybir.dt.float32
BF16 = mybir.dt.bfloat16

C = 256          # channels
D = 32           # q/k channels
NK = 4096        # keys per core (full sequence)
NQ = 2048        # queries per core (half sequence)
NJ = NK // 128   # 32 key tiles
NG = 4           # query groups
GI = 4           # i-tiles (128 queries) per group
ISPAN = NQ // NG  # 512 query columns per group
NCH = 8          # x arrives in 8 chunks of 512 columns
CHW = NK // NCH
EXP_SHIFT = -40.0

Exp = mybir.ActivationFunctionType.Exp
Copy = mybir.ActivationFunctionType.Copy
MULT = mybir.AluOpType.mult
ADD = mybir.AluOpType.add


def build(nc):
    x_bf = nc.declare_dram_parameter("x_bf", [C, NK], BF16, isOutput=False)
    xqT = nc.declare_dram_parameter("xqT", [NQ, C], F32, isOutput=False)
    # projection weights pre-arranged in SBUF layout [p, h, :] so each load
    # is one DMA with contiguous lines; wk separate (it gates the first
    # projection and rides the fast sync ring), wq|gamma*WvT packed on gpsimd
    wkp = nc.declare_dram_parameter("wkp", [128, 2, 128], BF16, isOutput=False)
    wqv = nc.declare_dram_parameter("wqv", [128, 2, 384], BF16, isOutput=False)
    bpack = nc.declare_dram_parameter("bpack", [128, 2], F32, isOutput=False)
    out_nc = nc.declare_dram_parameter("out_nc", [NQ, C], F32, isOutput=True)

    with tile.TileContext(nc) as tc:
        with (
            tc.tile_pool(name="singles", bufs=1) as singles,
            tc.tile_pool(name="epool", bufs=3) as e_pool,
            tc.tile_pool(name="osb", bufs=4) as osb_pool,
            tc.tile_pool(name="small", bufs=8) as small_pool,
            tc.tile_pool(name="s_ps", bufs=2, space="PSUM") as s_pool,
            tc.tile_pool(name="o_ps", bufs=4, space="PSUM") as o_pool,
        ):
            # PE warmup: the HAM clock gate keeps the tensor engine at half
            # clock until ~3.4us of sustained activity. Burn that window on
            # dummy matmuls over a memset tile while the x DMA is in flight
            # so the real projections run at full clock.
            wu_sb = singles.tile([128, 256], BF16)
            nc.gpsimd.memset(wu_sb, 0.0)
            for _ in range(12):
                ps_w = s_pool.tile([128, 128], F32, tag="ps_s", name="ps_wu")
                nc.tensor.matmul(
                    ps_w, wu_sb[:, 0:128], wu_sb[:, 0:128], start=True, stop=True
                )

            # ---------------- persistent SBUF inputs ----------------
            # wk leads the sync ring, then x chunks rotate over the
            # sync/scalar/gpsimd rings so all three drain in parallel. xqT
            # (residual, fp32) is only needed at the epilogues and queues
            # last on gpsimd.
            xf_sb = singles.tile([128, 2, NK], BF16)
            x_r = x_bf.rearrange("(h p) n -> p h n", p=128)

            def xsl(nch):
                return slice(nch * CHW, (nch + 1) * CHW)

            wk_sb = singles.tile([128, 2, 128], BF16)
            wqv_sb = singles.tile([128, 2, 384], BF16)
            bp_sb = singles.tile([128, 2], F32)
            nc.scalar.dma_start(out=wk_sb, in_=wkp[:, :, :])
            nc.gpsimd.dma_start(out=wqv_sb, in_=wqv[:, :, :])
            nc.gpsimd.dma_start(out=bp_sb, in_=bpack[:, :])
            # chunk 0 ships as two 256-col pieces so the first k-projection
            # can start half a chunk earlier
            nc.sync.dma_start(out=xf_sb[:, :, 0:256], in_=x_r[:, :, 0:256])
            nc.sync.dma_start(out=xf_sb[:, :, 256:512], in_=x_r[:, :, 256:512])
            for nch, eng in [
                (1, nc.scalar), (6, nc.gpsimd), (2, nc.sync),
                (3, nc.scalar), (7, nc.gpsimd), (4, nc.sync), (5, nc.scalar),
            ]:
                eng.dma_start(out=xf_sb[:, :, xsl(nch)], in_=x_r[:, :, xsl(nch)])
            wq_sb = wqv_sb[:, :, 0:128]
            wv_sb = wqv_sb[:, :, 128:384]
            bk_sb = bp_sb[:, 0:1]
            bq_sb = bp_sb[:, 1:2]

            xqT_sb = singles.tile([128, NQ // 128, C], F32)
            nc.gpsimd.dma_start(
                out=xqT_sb, in_=xqT.rearrange("(t p) c -> p t c", p=128)
            )

            shift_sb = singles.tile([128, 1], F32)
            nc.vector.memset(shift_sb, EXP_SHIFT)

            # ---------------- projections (chunk-pipelined) ----------------
            k_sb = singles.tile([128, NK], BF16)
            q_sb = singles.tile([128, NQ], BF16)
            vT_sb = singles.tile([128, NJ, C + 1], BF16)
            nc.vector.memset(vT_sb[:, :, C : C + 1], 1.0)

            for ch in range(NCH):
                # chunk 0 projects in two 256-col pieces to chase its split
                # DMA; the rest go in one 512-col piece
                ksl = (
                    [slice(0, 256), slice(256, 512)]
                    if ch == 0
                    else [xsl(ch)]
                )
                for sl in ksl:
                    ps = s_pool.tile(
                        [128, sl.stop - sl.start], F32, tag="ps_s", name="ps_kq"
                    )
                    for h in range(2):
                        nc.tensor.matmul(
                            ps,
                            wk_sb[:, h, :],
                            xf_sb[:, h, sl],
                            start=(h == 0),
                            stop=(h == 1),
                        )
                    if ch % 2 == 0:
                        nc.vector.tensor_scalar_add(k_sb[:, sl], ps, bk_sb)
                    else:
                        nc.scalar.add(k_sb[:, sl], ps, bk_sb)

                if ch < NQ // CHW:
                    ps = s_pool.tile([128, CHW], F32, tag="ps_s", name="ps_kq")
                    for h in range(2):
                        nc.tensor.matmul(
                            ps,
                            wq_sb[:, h, :],
                            xf_sb[:, h, xsl(ch)],
                            start=(h == 0),
                            stop=(h == 1),
                        )
                    if ch % 2 == 0:
                        nc.scalar.add(q_sb[:, xsl(ch)], ps, bq_sb)
                    else:
                        nc.vector.tensor_scalar_add(q_sb[:, xsl(ch)], ps, bq_sb)

                for j in range(ch * 4, ch * 4 + 4):
                    psv = o_pool.tile([128, C], F32, tag="ps_o", name="ps_v")
                    for h in range(2):
                        nc.tensor.matmul(
                            psv,
                            xf_sb[:, h, j * 128 : (j + 1) * 128],
                            wv_sb[:, h, :],
                            start=(h == 0),
                            stop=(h == 1),
                        )
                    # psum->sbuf copies split between vector and scalar so
                    # neither trails the PE during the projection phase
                    if j % 2 == 0:
                        nc.vector.tensor_copy(vT_sb[:, j, 0:C], psv)
                    else:
                        nc.scalar.mul(vT_sb[:, j, 0:C], psv, 1.0)

            # ---------------- attention ----------------
            # Flat software pipeline over (group, quad) steps at half-quad
            # granularity: while ACT runs exp on one half, the PE issues the
            # next step's packed score matmuls, then consumes the current
            # half with 8 attn matmuls. Scores prefetch crosses group
            # boundaries so the PE never drains at an epilogue.
            steps = [(g, q4) for g in range(NG) for q4 in range(NJ // 4)]
            score_tiles = {}
            ps_o_groups = {}

            def emit_scores_half(step, half):
                g, q4 = step
                isl = slice(g * ISPAN, (g + 1) * ISPAN)
                ps_s = s_pool.tile([128, 2, ISPAN], F32, tag="ps_s", name="ps_s")
                for rr in range(2):
                    r = half * 2 + rr
                    j = q4 * 4 + r
                    nc.tensor.matmul(
                        ps_s[:, rr, :],
                        k_sb[32 * r : 32 * (r + 1), j * 128 : (j + 1) * 128],
                        q_sb[32 * r : 32 * (r + 1), isl],
                        start=True,
                        stop=True,
                        tile_position=(32 * r, 0),
                    )
                score_tiles[(g, q4, half)] = ps_s

            emit_scores_half(steps[0], 0)
            emit_scores_half(steps[0], 1)
            for idx, (g, q4) in enumerate(steps):
                if q4 == 0:
                    ps_o_groups[g] = [
                        o_pool.tile([128, C + 1], F32, tag="ps_o", name="ps_o")
                        for _ in range(GI)
                    ]
                ps_o = ps_o_groups[g]
                nxt = steps[idx + 1] if idx + 1 < len(steps) else None
                for half in range(2):
                    ps_s = score_tiles.pop((g, q4, half))
                    e_sb = e_pool.tile(
                        [128, 2, ISPAN], BF16, tag="e_sb", name="e_sb"
                    )
                    nc.scalar.activation(
                        e_sb, ps_s, Exp, bias=shift_sb, scale=1.0
                    )
                    if nxt is not None:
                        emit_scores_half(nxt, half)
                    for rr in range(2):
                        r = half * 2 + rr
                        j = q4 * 4 + r
                        for t in range(GI):
                            nc.tensor.matmul(
                                ps_o[t],
                                e_sb[:, rr, t * 128 : (t + 1) * 128],
                                vT_sb[:, j, :],
                                start=(j == 0),
                                stop=(j == NJ - 1),
                            )
                if q4 == NJ // 4 - 1:
                    # Epilogue: f = out/denom + xr (gamma is folded into the
                    # host-side WvT and xr, so out already carries it).
                    # Reciprocals first on vector, then the wide mul+add work
                    # fans across vector (fused) and scalar+gpsimd so the
                    # psum buffers free fast and the last group drains fast.
                    # Everything rides vector here: an epilogue op on scalar
                    # would delay the next group's exp behind it in the FIFO.
                    for t in range(GI):
                        it = g * GI + t
                        r = small_pool.tile([128, 1], F32, tag="r", name="r")
                        nc.vector.reciprocal(r, ps_o[t][:, C : C + 1])
                        f_sb = osb_pool.tile([128, C], F32, tag="f_sb", name="f_sb")
                        nc.vector.scalar_tensor_tensor(
                            f_sb, ps_o[t][:, 0:C], r, xqT_sb[:, it, :],
                            op0=MULT, op1=ADD,
                        )
                        orows = out_nc[it * 128 : (it + 1) * 128, :]
                        if t < 3:
                            eng = (nc.sync, nc.scalar, nc.gpsimd)[t]
                            eng.dma_start(out=orows, in_=f_sb)
                        else:
                            nc.sync.dma_start(
                                out=orows[0:64, :], in_=f_sb[0:64, :]
                            )
                            nc.scalar.dma_start(
                                out=orows[64:128, :], in_=f_sb[64:128, :]
                            )
                    del ps_o_groups[g]
    return nc


def _install_trace_support():
    """Profiling-only plumbing for KERNEL_TRACE=1 runs: register the NTFF
    profile hook (this image's antenv lacks the axon_hooks shim) and keep
    trace artifacts local instead of uploading. Never used in plain runs."""
    import importlib.util
    import types

    import concourse.bass_utils as bu

    bu.upload_artifacts = lambda tmpdir: tmpdir
    if "antenv.axon_hooks" in sys.modules:
        return
    try:
        if importlib.util.find_spec("antenv.axon_hooks") is not None:
            return
    except (ValueError, ModuleNotFoundError):
        return
    import antenv
    from trn_agent_boot.trn_boot import _ntff_profile_via_ctypes

    mod = types.ModuleType("antenv.axon_hooks")
    mod._hook = _ntff_profile_via_ctypes("/opt/axon/libaxon_pjrt.so")
    mod.set_axon_ntff_profile_hook = lambda h: setattr(mod, "_hook", h)
    mod.get_axon_ntff_profile_hook = lambda: mod._hook
    sys.modules["antenv.axon_hooks"] = mod
    antenv.axon_hooks = mod


_cached = None


def _get_module():
    global _cached
    if _cached is None:
        nc = bacc.Bacc()
        build(nc)
        if not nc.is_finalized():
            nc.finalize()
        _cached = nc
    return _cached


def kernel(x, Wq, bq, Wk, bk, Wv, bv, gamma, **_unused):
    from concourse.bass_utils import run_bass_kernel_spmd
    import os

    B, Cx, H, W = x.shape
    N = H * W
    xf = np.ascontiguousarray(np.asarray(x, dtype=np.float32).reshape(B, Cx, N))
    Wq = np.asarray(Wq, np.float32)
    Wk = np.asarray(Wk, np.float32)
    Wv = np.asarray(Wv, np.float32)
    bq = np.asarray(bq, np.float32)
    bk = np.asarray(bk, np.float32)
    bv = np.asarray(bv, np.float32)
    gamma = np.asarray(gamma, np.float32)

    import ml_dtypes

    bf16 = ml_dtypes.bfloat16
    # weight pack in device SBUF layout [p, h, wk4 | wq4 | gamma*WvT]: Wk/Wq
    # tiled 4x along the output dim so one matmul writes all four 32-row
    # replicas; gamma folded into Wv so the epilogue needs no extra scale
    wkp = np.ascontiguousarray(
        np.tile(Wk.T, (1, 4)).reshape(2, 128, 128).transpose(1, 0, 2).astype(bf16)
    )
    wqv_cat = np.concatenate([np.tile(Wq.T, (1, 4)), gamma[0] * Wv.T], axis=1)
    wqv = np.ascontiguousarray(
        wqv_cat.reshape(2, 128, 384).transpose(1, 0, 2).astype(bf16)
    )
    bpack = np.ascontiguousarray(
        np.stack([np.tile(bk, 4), np.tile(bq, 4)], axis=1)
    )

    in_maps = []
    for core in range(8):
        b, half = core // 2, core % 2
        ioff = half * NQ
        xb = xf[b]
        x_bf = np.ascontiguousarray(np.roll(xb, -ioff, axis=1).astype(bf16))
        # residual with the V-projection bias folded in: the device computes
        # (e @ (gamma*v0))/denom + xqT, and gamma * e @ (v0 + bv) / denom
        # collapses to an extra gamma*bv[c] term
        xqT_np = np.ascontiguousarray(
            xb[:, ioff : ioff + NQ].T + gamma[0] * bv[None, :]
        )
        in_maps.append(
            {
                "x_bf": x_bf,
                "xqT": xqT_np,
                "wkp": wkp,
                "wqv": wqv,
                "bpack": bpack,
            }
        )

    nc = _get_module()
    trace = bool(int(os.environ.get("KERNEL_TRACE", "0")))
    if trace:
        _install_trace_support()
        tmpdir = os.environ.get("KERNEL_TRACE_DIR") or None
        res = run_bass_kernel_spmd(
            nc, in_maps, core_ids=list(range(8)), trace=True, tmpdir=tmpdir
        )
    else:
        res = run_bass_kernel_spmd(nc, in_maps, core_ids=list(range(8)))
    if trace and res.exec_time_ns is not None:
        print(f"HW exec time: {res.exec_time_ns} ns")
        print(f"HW exec time mean: {res.mean_exec_time_ns} ns")
        if res.instructions_and_trace is not None:
            print(f"trace: {res.instructions_and_trace[1]}")

    out = np.empty((B, Cx, N), np.float32)
    for core in range(8):
        b, half = core // 2, core % 2
        out[b][:, half * NQ : (half + 1) * NQ] = res.results[core]["out_nc"].T
    return out.reshape(B, Cx, H, W)
